# revision 1
# baseline (speedup 1.0000x reference)
"""CSPN (convolutional spatial propagation) step on 8 Trainium2 NeuronCores.

Computation (per batch element b, pixel (y, x)):
    out[b,0,y,x] = sum_{t=0..24} w[b,t,2+y,2+x] * src_t[b, y+2-t//5, x+2-t%5]
where src_t = h0 for the center tap (t=12) and hn otherwise, with zero
padding outside the image.

Sharding: B*H = 4*352 = 1408 output rows. Each core gets:
  - chunk A: one 128-row band  (batch c//2, rows 128*(c%2) .. +128)
  - chunk B: half of that batch's remaining 96-row band, split by columns
    (rows 256..352, cols 608*(c%2) .. +608)
so all 8 cores run an identical program on identically-shaped slices.

Device layout: H rows on SBUF partitions, W on the free dimension.  Row
(dy) shifts are pre-resolved on the host, which ships 5 row-shifted
copies of the padded source in per-partition-contiguous layouts; column
(dx) shifts are then free-dim offsets.  For 16-bit dtypes a second,
one-element-shifted copy of each source block keeps every window 4-byte
aligned so tensor_tensor runs in the 2x DVE perf mode.  Per tap group
(one dy row = 5 taps): multiply weight planes by shifted source
windows, pairwise-tree the 5 products, then tree the 5 group results.
DMAs are merged into a few large transfers and spread explicitly across
the two HWDGE rings (SP + ACT).
"""

import numpy as np

import concourse.bass as bass
import concourse.mybir as mybir
import concourse.tile as tile
from concourse.bass_utils import run_bass_kernel_spmd

K = 5
R = 2
B, H, W = 4, 352, 1216

# chunk name -> (partitions, out width, src block width)
CHUNKS = (
    ("B", 96, 608, 612),
    ("A", 128, 1216, 1220),
)

N_CORES = 8

# pool depths (A/B-tested on hardware)
PARITY = False  # ship a 2nd 1-elem-shifted source copy (2x mode for odd taps)
WP_BUFS = 2
SRCP_BUFS = 2
PP_BUFS = 2
STAGGERED = False  # staggered_reset on the bench For_i loop


def _split_drain_waits(nc):
    """walrus in this container accepts at most one sync-wait per
    instruction; move the extras onto NoOps placed just before it."""
    for bb in nc.main_func.blocks:
        insts = bb.bb.instructions if hasattr(bb, "bb") else bb.instructions
        i = 0
        while i < len(insts):
            ins = insts[i]
            if (
                ins.sync_info
                and ins.sync_info.on_wait
                and len(ins.sync_info.on_wait) > 1
            ):
                extras = ins.sync_info.on_wait[1:]
                ins.sync_info.on_wait = ins.sync_info.on_wait[:1]
                for j, wcond in enumerate(extras):
                    nop = mybir.InstNoOp(
                        name=f"{ins.name}-waitsplit-{j}",
                        ins=[],
                        outs=[],
                        engine=ins.engine,
                        sync_info=mybir.SyncInfo(on_wait=[wcond], on_update=[]),
                    )
                    insts.insert(i, nop)
                    i += 1
            i += 1


def _build_nc(np_dtype, repeat=1, bench=False):
    cdt = mybir.dt.float16 if np_dtype == np.float16 else mybir.dt.float32
    two_byte = np_dtype == np.float16
    npar = 2 if (two_byte and PARITY) else 1
    add = mybir.AluOpType.add
    mult = mybir.AluOpType.mult

    nc = bass.Bass()
    dram = {}
    if bench:
        dram["ident"] = nc.dram_tensor("ident", [128, 128], cdt)
    else:
        dram["ident"] = nc.declare_dram_parameter(
            "ident", [128, 128], cdt, isOutput=False
        )
    if bench:
        # timing variant: data lives in internal (uninitialized) DRAM so each
        # call ships ~nothing over the wire; tiny external tensors for binding
        dram["_in"] = nc.declare_dram_parameter("_in", [1, 128], cdt, isOutput=False)
        dram["_out"] = nc.declare_dram_parameter("_out", [1, 128], cdt, isOutput=True)
    for nm, P, Wd, WBLK in CHUNKS:
        # host pre-gathers everything so every DMA is contiguous per
        # partition:  w[g, p, dx, x];  src1[p, blk(dy 0-1), c];
        # src2[p, blk(dy 2-4), c] with h0 appended as a final Wd block.
        names_shapes = (
            ("w" + nm, [K, P, K, Wd]),
            ("src1" + nm, [P, 2 * npar * WBLK]),
            ("src2" + nm, [P, 3 * npar * WBLK + Wd]),
        )
        for tname, tshape in names_shapes:
            if bench:
                dram[tname] = nc.dram_tensor(tname, tshape, cdt)
            else:
                dram[tname] = nc.declare_dram_parameter(
                    tname, tshape, cdt, isOutput=False
                )
        if bench:
            dram["out" + nm] = nc.dram_tensor("out" + nm, [P, Wd], cdt)
        else:
            dram["out" + nm] = nc.declare_dram_parameter(
                "out" + nm, [P, Wd], cdt, isOutput=True
            )

    pool_bufs = 2 if two_byte else 1
    with tile.TileContext(nc) as tc:
        with (
            tc.tile_pool(name="srcp", bufs=SRCP_BUFS if two_byte else 1) as srcp,
            tc.tile_pool(name="wp", bufs=WP_BUFS if two_byte else 1) as wp,
            tc.tile_pool(name="wps", bufs=3 if two_byte else 1) as wps,
            tc.tile_pool(name="pp", bufs=PP_BUFS if two_byte else 1) as pp,
            tc.tile_pool(name="psp", bufs=1, space="PSUM") as psp,
            tc.tile_pool(name="cstp", bufs=1) as cstp,
            tc.tile_pool(name="accp", bufs=pool_bufs) as accp,
        ):
            ident = cstp.tile([128, 128], cdt, name="ident")
            nc.sync.dma_start(ident[:], dram["ident"][:])

            def emit_body():
                for nm, P, Wd, WBLK in CHUNKS:
                    # ring roles alternate per chunk to balance total bytes
                    e1, e2 = (nc.sync, nc.scalar) if nm == "A" else (nc.scalar, nc.sync)
                    # ---- loads ----------------------------------------------
                    # st1: dy 0-1 source blocks; st2: dy 2-4 blocks + h0
                    st1 = srcp.tile([128, 2, npar, WBLK], cdt, tag="st1", name="st1")
                    n1 = 2 * npar * WBLK
                    e1.dma_start(
                        st1[0:P].rearrange("p a b c -> p (a b c)"),
                        bass.AP(dram["src1" + nm], 0, [[n1, P], [1, n1]]),
                    )
                    st2 = srcp.tile(
                        [128, 3 * npar * WBLK + Wd], cdt, tag="st2", name="st2"
                    )
                    n2 = 3 * npar * WBLK + Wd
                    e2.dma_start(
                        st2[0:P],
                        bass.AP(dram["src2" + nm], 0, [[n2, P], [1, n2]]),
                    )

                    def win(dy, par, c0, nstep, n):
                        """source window AP: [p][tap j: step nstep][x: Wd]
                        at column c0 of block (dy, par)."""
                        if dy < 2:
                            t, off = st1, (dy * npar + par) * WBLK + c0
                        else:
                            t, off = st2, ((dy - 2) * npar + par) * WBLK + c0
                        return bass.AP(
                            t.tensor,
                            t.offset + off,
                            [[t.ap[0][0], P], [nstep, n], [1, Wd]],
                        )

                    ht = st2[0:P, 3 * npar * WBLK : 3 * npar * WBLK + Wd]

                    # ---- weights: pair DMAs {0,1}, {2,3}, {4} ---------------
                    wmap = []
                    for gi, (g0, ng) in enumerate(((0, 2), (2, 2), (4, 1))):
                        wt = wps.tile([128, 2, K, Wd], cdt, tag="wts", name="wt")
                        w_in = bass.AP(
                            dram["w" + nm],
                            g0 * P * K * Wd,
                            [[K * Wd, P], [P * K * Wd, ng], [1, K * Wd]],
                        )
                        eng = (e2, e1, e2)[gi]
                        eng.dma_start(
                            wt[0:P, 0:ng].rearrange("p a b c -> p a (b c)"), w_in
                        )
                        for sub in range(ng):
                            wmap.append(wt[:, sub])

                    nbank = (Wd + 511) // 512
                    ps = psp.tile(
                        [128, nbank * 512],
                        mybir.dt.float32,
                        tag="ps" + nm,
                        name="ps",
                    )
                    ccs = []
                    for cc in range(nbank):
                        c0 = cc * 512
                        ccs.append((c0, min(512, Wd - c0)))
                    for dy in range(K):
                        wt = wmap[dy]
                        pt = pp.tile([128, K, Wd], cdt, tag="pt", name="pt")
                        # odd taps (dx 1,3 -> ox 3,1): with PARITY they read
                        # the 1-elem-shifted copy 4B-aligned (2x mode); without
                        # it they read the base copy misaligned (1x mode)
                        opar, oc0 = (1, 2) if npar == 2 else (0, 3)
                        # ---- products: pt[:, dx, :] = w[5dy+dx] * win(ox=4-dx)
                        if two_byte:
                            if dy != 2:
                                nc.vector.tensor_tensor(
                                    pt[0:P, 0:5:2, :],
                                    wt[0:P, 0:5:2, :],
                                    win(dy, 0, 4, -2, 3),
                                    mult,
                                )
                            else:
                                nc.vector.tensor_tensor(
                                    pt[0:P, 0:5:4, :],
                                    wt[0:P, 0:5:4, :],
                                    win(dy, 0, 4, -4, 2),
                                    mult,
                                )
                                nc.vector.tensor_tensor(
                                    pt[0:P, 2, :], wt[0:P, 2, :], ht, mult
                                )
                            nc.vector.tensor_tensor(
                                pt[0:P, 1:4:2, :],
                                wt[0:P, 1:4:2, :],
                                win(dy, opar, oc0, -2, 2),
                                mult,
                            )
                        else:
                            if dy != 2:
                                nc.vector.tensor_tensor(
                                    pt[0:P, :, :],
                                    wt[0:P, :, :],
                                    win(dy, 0, 4, -1, K),
                                    mult,
                                )
                            else:
                                nc.vector.tensor_tensor(
                                    pt[0:P, 0:2, :],
                                    wt[0:P, 0:2, :],
                                    win(dy, 0, 4, -1, 2),
                                    mult,
                                )
                                nc.vector.tensor_tensor(
                                    pt[0:P, 3:5, :],
                                    wt[0:P, 3:5, :],
                                    win(dy, 0, 1, -1, 2),
                                    mult,
                                )
                                nc.vector.tensor_tensor(
                                    pt[0:P, 2, :], wt[0:P, 2, :], ht, mult
                                )
                        # ---- accumulate the 5 products into PSUM on PE:
                        # ps[:, x] += I.T @ pt[:, dx, x]  (fp32 accumulate)
                        for dx in range(K):
                            for c0, cn in ccs:
                                nc.tensor.matmul(
                                    ps[0:P, c0 : c0 + cn],
                                    ident[0:P, 0:P],
                                    pt[0:P, dx, c0 : c0 + cn],
                                    start=(dy == 0 and dx == 0),
                                    stop=(dy == K - 1 and dx == K - 1),
                                    skip_group_check=True,
                                )

                    # ---- evict PSUM -> SBUF fp16 on ACT, then store ---------
                    at = accp.tile([128, Wd], cdt, tag="at", name="at")
                    nc.scalar.activation(
                        at[0:P, :],
                        ps[0:P, 0:Wd],
                        mybir.ActivationFunctionType.Copy,
                    )
                    e1.dma_start(dram["out" + nm][:], at[0:P])

            if bench:
                with tc.For_i(0, repeat, 1, staggered_reset=STAGGERED):
                    emit_body()
                dumt = accp.tile([1, 128], cdt, tag="dumt", name="dumt")
                nc.vector.memset(dumt[:], 0.0)
                nc.sync.dma_start(dram["_out"][:], dumt[:])
            else:
                for _rep in range(repeat):
                    emit_body()

    _split_drain_waits(nc)
    return nc


def _host_prep(guide_weight, hn, h0, np_dtype):
    """Slice, pad, and pre-gather the full inputs into the 8 per-core input
    maps, in the exact DMA-friendly layouts the device program expects."""
    npar = 2 if (np_dtype == np.float16 and PARITY) else 1
    gw = np.asarray(guide_weight)
    hnp = np.zeros((B, H + 2 * R, W + 2 * R + 2), dtype=np_dtype)
    hnp[:, R : R + H, R : R + W] = np.asarray(hn)[:, 0]
    h0c = np.asarray(h0)[:, 0].astype(np_dtype)

    def prep_w(b, row0, col0, P, Wd):
        sl = gw[b, :, R + row0 : R + row0 + P, R + col0 : R + col0 + Wd]
        sl = sl.astype(np_dtype).reshape(K, K, P, Wd).transpose(0, 2, 1, 3)
        return np.ascontiguousarray(sl)

    def prep_src(b, row0, col0, P, Wd, WBLK):
        """dy-ordered shifted source blocks; h0 appended after dy 2-4."""
        s1 = np.empty((P, 2 * npar, WBLK), dtype=np_dtype)
        s2 = np.empty((P, 3 * npar * WBLK + Wd), dtype=np_dtype)
        for dy in range(K):
            oy = 2 * R - dy
            for par in range(npar):
                blkdata = hnp[
                    b, row0 + oy : row0 + oy + P, col0 + par : col0 + par + WBLK
                ]
                if dy < 2:
                    s1[:, dy * npar + par, :] = blkdata
                else:
                    j = ((dy - 2) * npar + par) * WBLK
                    s2[:, j : j + WBLK] = blkdata
        s2[:, 3 * npar * WBLK :] = h0c[b, row0 : row0 + P, col0 : col0 + Wd]
        return np.ascontiguousarray(s1.reshape(P, -1)), s2

    in_maps = []
    for c in range(N_CORES):
        bA, yA = c // 2, 128 * (c % 2)
        bB, colB = c // 2, 608 * (c % 2)
        sA1, sA2 = prep_src(bA, yA, 0, 128, 1216, 1220)
        sB1, sB2 = prep_src(bB, 256, colB, 96, 608, 612)
        in_maps.append(
            {
                "ident": np.eye(128, dtype=np_dtype),
                "wA": prep_w(bA, yA, 0, 128, 1216),
                "src1A": sA1,
                "src2A": sA2,
                "wB": prep_w(bB, 256, colB, 96, 608),
                "src1B": sB1,
                "src2B": sB2,
            }
        )
    return in_maps


def _assemble(results):
    out = np.zeros((B, 1, H, W), dtype=np.float32)
    for c in range(N_CORES):
        bA, yA = c // 2, 128 * (c % 2)
        bB, colB = c // 2, 608 * (c % 2)
        out[bA, 0, yA : yA + 128, :] = results[c]["outA"].astype(np.float32)
        out[bB, 0, 256:352, colB : colB + 608] = results[c]["outB"].astype(np.float32)
    return out


_NC_CACHE = {}


def _get_nc(np_dtype, repeat=1, bench=False):
    key = (np.dtype(np_dtype).name, repeat, bench)
    if key not in _NC_CACHE:
        _NC_CACHE[key] = _build_nc(np_dtype, repeat, bench)
    return _NC_CACHE[key]


def run_on_cores(in_maps, np_dtype, repeat=1, bench=False):
    nc = _get_nc(np_dtype, repeat, bench)
    return run_bass_kernel_spmd(nc, in_maps, list(range(N_CORES)), trace=False)


def kernel(guide_weight, hn, h0, _dtype=np.float16):
    in_maps = _host_prep(guide_weight, hn, h0, np.dtype(_dtype))
    res = run_on_cores(in_maps, np.dtype(_dtype))
    return _assemble(res.results)

